# revision 16
# baseline (speedup 1.0000x reference)
"""Trainium2 Bass kernel for nn_CorrTorch_unfold (B=1, C=32, D=32, H=W=128).

Math (validated in proto2.py): with p = h' = 32t + wb, i the sigma-permuted
channel index (cp = sigma(i) = 25i mod 32):

  OS[p, i*36 + (3kh+kw)*4 + j] = sum_c A[p, kh, i, kw + 32j + c]
                                        * XTW[p, i + 3kh + kw, 32j + c]
  A[p, kh, i, w]   = y_pad[sigma(i), d, p + kh, w]       (host-permuted DMA)
  XTW[p, s, j*32+c] = x[c, 4*(s%32) + t, 4wb + j]        (PE transposes)

Because 9*sigma(i) = i (mod 32), the XT slot index is affine in i, so ONE
custom-DVE multiply-scan instruction covers all 32 i per (kh, kw): the scan
runs a prefix sum of products over the 4096-long (i, w) chain; 32-aligned
prefix differences recover the c-group sums (zero column per chunk seeds
group 0). 9 scans + 3 diff TTs per depth slice replace the baseline's 96+8.

Everything is fp16 so the scan can use the DVE 2x_1p perf mode: a hand-built
uops_2x program (registered alongside the stock 1x lowering, perf_max=1)
computes state += lo*lo' + hi*hi' per cycle — 2 products/cycle. Only
odd-position prefixes are read (group ends land at chain index 31 mod 32),
so the pair-granular state is exact there. kw=1 offsets A by one fp16
element (2B, breaking the 4B pairing alignment), so a shifted copy A1 is
built on the otherwise-idle GPSIMD engine.

Sharding: D=32 depth slices, 4 per core across 8 cores (slice d needs only
y slice d-1). leaky_relu + the (i,k,j) -> (k2,d,h2,w2) unpermute run on host.
"""
import numpy as np

_PROG_CACHE = {}
_RUN_OPTS = {"trace": False}
_LAST_RESULT = {}

D_LOC = 4
N_CORES = 8
C = 32
H = W = 128
WPAD = 130
NXT = 40            # XT slots: i + 3kh + kw <= 31 + 8, slots 32..39 dup 0..7
SCRW = 4104         # per-(kw) scan chunk: col 1 zero, cols 2..4097 written
USE_2X = True       # hand-built 2x_1p uop program for the scan
A1_ENGINE = "act"   # engine that builds the kw=1 shifted copy of A

OPNAME = "MUL_SCAN2X_ANT"


def _register_mul_scan2x():
    """Custom DVE op: out = prefix-sum over free dim of in0*in1, with a
    2x_1p uop variant (2 fp16 products/cycle; odd-position prefixes exact)."""
    import numpy as np
    import concourse.dve_ops as dve_ops
    from concourse.dve_spec import Spec, Src0, Src1, AluOp, scan, lower
    from concourse.dve_uop import (
        DveOpSpec, UopConfig, UopDpConfig, InpSel, AluInp, DelayInp,
        OutPath, OutSel, Trigger, ENABLE,
    )

    for o in dve_ops.OPS:
        if o.name == OPNAME:
            return o

    def _ref(in0, in1, c0, c1, c2):
        p = in0.shape[0]
        prod = (np.asarray(in0, np.float32).reshape(p, -1) *
                np.asarray(in1, np.float32).reshape(p, -1))
        return np.add.accumulate(prod, axis=1).reshape(in0.shape)

    spec = Spec(body=scan(AluOp.ADD, Src0 * Src1), reference=_ref)
    uops_1x = lower(spec, ver="v3")

    # --- 2x_1p program: seed uop zeroes stage-3 accumulator, steady uop
    # computes p_lo (st0), p_hi (st1), p_lo+p_hi (st2), state+=psum (st3).
    u0 = UopConfig()
    u0.enable_input(InpSel.ZERO, 3)                       # lane3 -> PREV_DELAY_2
    u0.repeat_count = 1
    u0.trigger = (Trigger.COUNT, Trigger.NONE, Trigger.NONE)
    u0.next_uop = (1, 0, 0)
    for k in range(3):
        u0.datapath_config[k] = UopDpConfig().pass_through_delay(2)
    u0.datapath_config[3] = UopDpConfig().enable_alu(
        AluOp.BYPASS, AluInp.PREV_DELAY_2, AluInp.PREV_DELAY_2)
    for k in range(4, 8):
        u0.datapath_config[k] = UopDpConfig().pass_through_alu()

    u1 = UopConfig()
    u1.enable_input(InpSel.SRC_0, 1).enable_input(InpSel.SRC_1, 2)
    u1.enable_input(InpSel.SRC_0_HI, 3).enable_input(InpSel.SRC_1_HI, 4)
    u1.require_inp0 = ENABLE
    u1.require_inp1 = ENABLE
    u1.trigger = (Trigger.SRC_TENSOR_DONE, Trigger.NONE, Trigger.NONE)
    u1.next_uop = (0, 0, 0)
    u1.datapath_config[0] = UopDpConfig().enable_alu(
        AluOp.MULTIPLY, AluInp.PREV_DELAY_0, AluInp.PREV_DELAY_1
    ).pass_through_delay(2, 3)
    u1.datapath_config[1] = UopDpConfig().enable_alu(
        AluOp.MULTIPLY, AluInp.PREV_DELAY_2, AluInp.PREV_DELAY_3
    ).enable_delay_from_src(DelayInp.PREV_ALU_OUT, 0)
    u1.datapath_config[2] = UopDpConfig().enable_alu(
        AluOp.ADD, AluInp.PREV_ALU_OUT, AluInp.PREV_DELAY_0)
    u1.datapath_config[3] = UopDpConfig().enable_alu(
        AluOp.ADD, AluInp.CURR_ALU_OUT, AluInp.PREV_ALU_OUT)
    for k in range(4, 8):
        u1.datapath_config[k] = UopDpConfig().pass_through_alu()
    u1.enable_output(OutSel.ALU_OUT, OutPath.WR0_LO)
    u1.enable_output(OutSel.ALU_OUT, OutPath.WR0_HI)

    row = 1 + len(dve_ops.OPS)
    assert row < 0x20
    spec2x = DveOpSpec(
        name=OPNAME, opcode=row, uops=uops_1x,
        uops_2x=[u0, u1] if USE_2X else None,
        perf_max=1 if USE_2X else 0, rd1_en=True)
    spec2x.validate("v3")
    shas = {"v3": spec2x.sha("v3")}

    class _DveOp2x(dve_ops.DveOp):
        def compile(self, ver):
            assert ver == "v3", ver
            return spec2x

    op = _DveOp2x(OPNAME, spec, subdim=False, uops_sha=shas)
    dve_ops.OPS.append(op)
    dve_ops.CUSTOM_DVE_SPECS[op.name] = spec
    dve_ops._SUB_OPCODE_FOR_NAME[op.name] = row
    return op


def _build_program():
    import concourse.bass as bass
    import concourse.bacc as bacc
    import concourse.mybir as mybir
    from concourse.tile import TileContext
    from bass_rust import VecI64Pair

    mul_scan = _register_mul_scan2x()

    f32 = mybir.dt.float32
    f16 = mybir.dt.float16

    def apv(base_ap, offset, dims):
        a = base_ap.copy()
        part = list(a.ap[0])
        a.ap = VecI64Pair([part] + [list(d) for d in dims])
        a.offset = a.offset + offset
        return a

    nc = bacc.Bacc()
    # per-core inputs (fp16): x slab [d,c,h,w]; y slab pre-shifted+padded,
    # channel-permuted by sigma(i)=25i%32: [d, i, 130, 130]
    x_in = nc.dram_tensor("xin", [D_LOC, C, H, W], f16, kind="ExternalInput")
    y_in = nc.dram_tensor("yin", [D_LOC, 128, 3 * C * WPAD], f16,
                          kind="ExternalInput")
    a1_in = nc.dram_tensor("a1in", [D_LOC, 128, 3 * C * 128], f16,
                           kind="ExternalInput")
    ident = nc.dram_tensor("ident", [32, 32], f16, kind="ExternalInput")
    out = nc.dram_tensor("out", [D_LOC, 128, 1152], f16,
                         kind="ExternalOutput")

    with TileContext(nc) as tc:
        with tc.tile_pool(name="const", bufs=1) as cpool, \
             tc.tile_pool(name="a", bufs=2) as apool, \
             tc.tile_pool(name="a1", bufs=2) as a1pool, \
             tc.tile_pool(name="xn", bufs=1) as xpool, \
             tc.tile_pool(name="xt", bufs=2) as xtpool, \
             tc.tile_pool(name="os", bufs=2) as ospool, \
             tc.tile_pool(name="ps", bufs=4, space="PSUM") as pspool:

            idt = cpool.tile([32, 32], f16)
            nc.sync.dma_start(idt[:], ident[:])

            # scan scratch: 2 banks x 3 kw-chunks of SCRW (banks alternate by
            # kh so scans of kh+1 never wait on diffs of kh); cols
            # bank*3*SCRW + kw*SCRW + {0,1} stay zero
            SCR = cpool.tile([128, 6 * SCRW], f16)
            zc = apv(SCR[:], 0, [[SCRW, 6], [1, 2]])
            nc.gpsimd.memset(zc, 0.0)

            def emit_loads(d):
                # x slice load first, in quarters: PE transposes of
                # quarter q start as soon as that quarter lands
                xf = xpool.tile([C, H * W], f16)
                for q in range(4):
                    nc.sync.dma_start(
                        xf[:, 4096 * q:4096 * (q + 1)],
                        x_in[d, :, 32 * q:32 * (q + 1), :].rearrange(
                            "c h w -> c (h w)"))
                # A[p, kh*4160 + i*130 + w] = y_pad[sigma(i), d, p+kh, w],
                # pre-arranged on host: one contiguous DMA
                A = apool.tile([128, 3 * C * WPAD], f16)
                nc.sync.dma_start(A[:], y_in[d])
                # A1[p, kh*4096 + i*128 + w] = A[p, kh*4160 + i*130 + 1 + w],
                # pre-arranged on host (4B-aligned rows for the kw=1 scans)
                A1 = a1pool.tile([128, 3 * C * 128], f16)
                nc.sync.dma_start(A1[:], a1_in[d])
                return xf, A, A1

            def emit_build(d, xf, A):
                # XTW via PE transposes; PSUM -> SBUF fp16 via ACT
                XTW = xtpool.tile([128, NXT * 128], f16)
                for q in range(4):
                    for b in range(2):
                        PT = pspool.tile([128, 512], f16)
                        for mi in range(4):
                            m = 8 * q + 4 * b + mi
                            for j in range(4):
                                tin = apv(xf[:], 4 * m * 128 + j,
                                          [[128, 4], [4, 32]])
                                nc.tensor.transpose(
                                    PT[:, mi * 128 + j * 32:
                                       mi * 128 + (j + 1) * 32], tin, idt[:])
                        m0 = 8 * q + 4 * b
                        nc.scalar.copy(
                            XTW[:, m0 * 128:(m0 + 4) * 128], PT[:])
                        if m0 < 8:  # dup slots 32..39 <- m 0..7
                            nc.scalar.copy(
                                XTW[:, (32 + m0) * 128:(32 + m0 + 4) * 128],
                                PT[:])
                return XTW

            def emit_compute(d, A, A1, XTW):
                OS = ospool.tile([128, 1152], f16)
                for kh in range(3):
                    bank = ((d * 3 + kh) % 2) * 3 * SCRW
                    for kw in (0, 2, 1):
                        if kw == 1:
                            in0 = apv(A1[:], kh * C * 128,
                                      [[128, C], [1, 128]])
                        else:
                            in0 = apv(A[:], kh * C * WPAD + kw,
                                      [[WPAD, C], [1, 128]])
                        in1 = apv(XTW[:], (3 * kh + kw) * 128,
                                  [[128, C], [1, 128]])
                        o = apv(SCR[:], bank + kw * SCRW + 2, [[1, 4096]])
                        inst = nc.vector._custom_dve(mul_scan, out=o,
                                                     in0=in0, in1=in1)
                        if USE_2X:
                            inst.ins.perf_max = 1
                    # group sums = prefix diffs; ends chunk+33+32g, starts
                    # chunk+1+32g (col chunk+1 is the memset zero column)
                    ends = apv(SCR[:], bank + 33,
                               [[SCRW, 3], [128, C], [32, 4]])
                    starts = apv(SCR[:], bank + 1,
                                 [[SCRW, 3], [128, C], [32, 4]])
                    od = apv(OS[:], kh * 12, [[4, 3], [36, C], [1, 4]])
                    nc.vector.tensor_tensor(od, ends, starts,
                                            mybir.AluOpType.subtract)
                # out store on the GpSimd DGE queue so it never blocks the
                # SP queue's input loads for the next slice
                nc.gpsimd.dma_start(out[d], OS[:])

            pending = None
            for d in range(D_LOC):
                xf, A, A1 = emit_loads(d)
                XTW = emit_build(d, xf, A)
                if pending is not None:
                    emit_compute(*pending)
                pending = (d, A, A1, XTW)
            emit_compute(*pending)

    nc.finalize()
    return nc


def _get_program():
    if "nc" not in _PROG_CACHE:
        _PROG_CACHE["nc"] = _build_program()
    return _PROG_CACHE["nc"]


_SIGMA = [(25 * i) % 32 for i in range(32)]      # channel stored at block i
_INV9 = [(9 * cp) % 32 for cp in range(32)]      # i such that sigma(i)=cp


def kernel(x: np.ndarray, y: np.ndarray) -> np.ndarray:
    from concourse.bass_utils import run_bass_kernel_spmd

    x = np.asarray(x)
    y = np.asarray(y)
    B, C_, D, H_, W_ = x.shape
    assert (B, C_, D, H_, W_) == (1, 32, 32, 128, 128)

    # host prep: fp16, depth-shifted + H/W-padded + sigma-permuted y,
    # pre-arranged into the on-chip A layout [d, p, kh*4160 + i*130 + w]
    y_sp = np.zeros((D, C_, WPAD, WPAD), np.float16)
    y_perm = y[0].transpose(1, 0, 2, 3)[:, _SIGMA]      # [d, i, h, w]
    y_sp[1:, :, 1:129, 1:129] = y_perm[:-1].astype(np.float16)
    y_hp = y_sp.transpose(0, 2, 1, 3)                   # [d, hrow, i, w]
    y_A = np.empty((D, 128, 3, C_, WPAD), np.float16)
    for kh in range(3):
        y_A[:, :, kh] = y_hp[:, kh:kh + 128]
    y_A = y_A.reshape(D, 128, 3 * C_ * WPAD)
    y_A1 = np.ascontiguousarray(
        y_A.reshape(D, 128, 3, C_, WPAD)[..., 1:129]).reshape(
        D, 128, 3 * C_ * 128)
    x_d = np.ascontiguousarray(
        x[0].transpose(1, 0, 2, 3).astype(np.float16))  # [d, c, h, w]
    id_np = np.eye(32, dtype=np.float16)

    nc = _get_program()
    in_maps = [
        {"xin": x_d[4 * j:4 * j + 4],
         "yin": y_A[4 * j:4 * j + 4],
         "a1in": y_A1[4 * j:4 * j + 4],
         "ident": id_np}
        for j in range(N_CORES)
    ]
    res = run_bass_kernel_spmd(nc, in_maps, core_ids=list(range(N_CORES)),
                               trace=_RUN_OPTS["trace"])
    _LAST_RESULT["res"] = res
    packed = np.concatenate([res.results[j]["out"] for j in range(N_CORES)],
                            axis=0).astype(np.float32)  # [32, 128, 1152]

    # host unpermute [d, p=(t,wb), i*36+k*4+j] -> [1, 9, D, H, W] + leaky
    a = packed.reshape(D, 4, 32, 32, 9, 4)               # d t wb i k j
    a = a[:, :, :, _INV9]                                # d t wb cp k j
    a = a.transpose(3, 4, 0, 1, 2, 5)                    # cp k d t wb j
    a = np.ascontiguousarray(a).reshape(9, 32, D, 4, 32, 4)  # k2 m d t wb j
    a = a.transpose(0, 2, 1, 3, 4, 5)                    # k2 d m t wb j
    a = np.ascontiguousarray(a).reshape(9, D, 128, 128)
    out = a[None]
    return np.where(out >= 0, out, np.float32(0.2) * out).astype(np.float32)


# revision 17
# speedup vs baseline: 1.0900x; 1.0900x over previous
"""Trainium2 Bass kernel for nn_CorrTorch_unfold (B=1, C=32, D=32, H=W=128).

Math (validated in proto2.py): with p = h' = 32t + wb, i the sigma-permuted
channel index (cp = sigma(i) = 25i mod 32):

  OS[p, i*36 + (3kh+kw)*4 + j] = sum_c A[p, kh, i, kw + 32j + c]
                                        * XTW[p, i + 3kh + kw, 32j + c]
  A[p, kh, i, w]   = y_pad[sigma(i), d, p + kh, w]       (host-permuted DMA)
  XTW[p, s, j*32+c] = x[c, 4*(s%32) + t, 4wb + j]        (PE transposes)

Because 9*sigma(i) = i (mod 32), the XT slot index is affine in i, so ONE
custom-DVE multiply-scan instruction covers all 32 i per (kh, kw): the scan
runs a prefix sum of products over the 4096-long (i, w) chain; 32-aligned
prefix differences recover the c-group sums (zero column per chunk seeds
group 0). 9 scans + 3 diff TTs per depth slice replace the baseline's 96+8.

Everything is fp16 so the scan can use the DVE 2x_1p perf mode: a hand-built
uops_2x program (registered alongside the stock 1x lowering, perf_max=1)
computes state += lo*lo' + hi*hi' per cycle — 2 products/cycle. Only
odd-position prefixes are read (group ends land at chain index 31 mod 32),
so the pair-granular state is exact there. kw=1 offsets A by one fp16
element (2B, breaking the 4B pairing alignment), so a shifted copy A1 is
built on the otherwise-idle GPSIMD engine.

Sharding: D=32 depth slices, 4 per core across 8 cores (slice d needs only
y slice d-1). leaky_relu + the (i,k,j) -> (k2,d,h2,w2) unpermute run on host.
"""
import numpy as np

_PROG_CACHE = {}
_RUN_OPTS = {"trace": False}
_LAST_RESULT = {}

D_LOC = 4
N_CORES = 8
C = 32
H = W = 128
WPAD = 130
NXT = 40            # XT slots: i + 3kh + kw <= 31 + 8, slots 32..39 dup 0..7
SCRW = 4104         # per-(kw) scan chunk: col 1 zero, cols 2..4097 written
USE_2X = True       # hand-built 2x_1p uop program for the scan
A1_ENGINE = "act"   # engine that builds the kw=1 shifted copy of A

OPNAME = "MUL_SCAN2X_ANT"


def _register_mul_scan2x():
    """Custom DVE op: out = prefix-sum over free dim of in0*in1, with a
    2x_1p uop variant (2 fp16 products/cycle; odd-position prefixes exact)."""
    import numpy as np
    import concourse.dve_ops as dve_ops
    from concourse.dve_spec import Spec, Src0, Src1, AluOp, scan, lower
    from concourse.dve_uop import (
        DveOpSpec, UopConfig, UopDpConfig, InpSel, AluInp, DelayInp,
        OutPath, OutSel, Trigger, ENABLE,
    )

    for o in dve_ops.OPS:
        if o.name == OPNAME:
            return o

    def _ref(in0, in1, c0, c1, c2):
        p = in0.shape[0]
        prod = (np.asarray(in0, np.float32).reshape(p, -1) *
                np.asarray(in1, np.float32).reshape(p, -1))
        return np.add.accumulate(prod, axis=1).reshape(in0.shape)

    spec = Spec(body=scan(AluOp.ADD, Src0 * Src1), reference=_ref)
    uops_1x = lower(spec, ver="v3")

    # --- 2x_1p program: seed uop zeroes stage-3 accumulator, steady uop
    # computes p_lo (st0), p_hi (st1), p_lo+p_hi (st2), state+=psum (st3).
    u0 = UopConfig()
    u0.enable_input(InpSel.ZERO, 3)                       # lane3 -> PREV_DELAY_2
    u0.repeat_count = 1
    u0.trigger = (Trigger.COUNT, Trigger.NONE, Trigger.NONE)
    u0.next_uop = (1, 0, 0)
    for k in range(3):
        u0.datapath_config[k] = UopDpConfig().pass_through_delay(2)
    u0.datapath_config[3] = UopDpConfig().enable_alu(
        AluOp.BYPASS, AluInp.PREV_DELAY_2, AluInp.PREV_DELAY_2)
    for k in range(4, 8):
        u0.datapath_config[k] = UopDpConfig().pass_through_alu()

    u1 = UopConfig()
    u1.enable_input(InpSel.SRC_0, 1).enable_input(InpSel.SRC_1, 2)
    u1.enable_input(InpSel.SRC_0_HI, 3).enable_input(InpSel.SRC_1_HI, 4)
    u1.require_inp0 = ENABLE
    u1.require_inp1 = ENABLE
    u1.trigger = (Trigger.SRC_TENSOR_DONE, Trigger.NONE, Trigger.NONE)
    u1.next_uop = (0, 0, 0)
    u1.datapath_config[0] = UopDpConfig().enable_alu(
        AluOp.MULTIPLY, AluInp.PREV_DELAY_0, AluInp.PREV_DELAY_1
    ).pass_through_delay(2, 3)
    u1.datapath_config[1] = UopDpConfig().enable_alu(
        AluOp.MULTIPLY, AluInp.PREV_DELAY_2, AluInp.PREV_DELAY_3
    ).enable_delay_from_src(DelayInp.PREV_ALU_OUT, 0)
    u1.datapath_config[2] = UopDpConfig().enable_alu(
        AluOp.ADD, AluInp.PREV_ALU_OUT, AluInp.PREV_DELAY_0)
    u1.datapath_config[3] = UopDpConfig().enable_alu(
        AluOp.ADD, AluInp.CURR_ALU_OUT, AluInp.PREV_ALU_OUT)
    for k in range(4, 8):
        u1.datapath_config[k] = UopDpConfig().pass_through_alu()
    u1.enable_output(OutSel.ALU_OUT, OutPath.WR0_LO)
    u1.enable_output(OutSel.ALU_OUT, OutPath.WR0_HI)

    row = 1 + len(dve_ops.OPS)
    assert row < 0x20
    spec2x = DveOpSpec(
        name=OPNAME, opcode=row, uops=uops_1x,
        uops_2x=[u0, u1] if USE_2X else None,
        perf_max=1 if USE_2X else 0, rd1_en=True)
    spec2x.validate("v3")
    shas = {"v3": spec2x.sha("v3")}

    class _DveOp2x(dve_ops.DveOp):
        def compile(self, ver):
            assert ver == "v3", ver
            return spec2x

    op = _DveOp2x(OPNAME, spec, subdim=False, uops_sha=shas)
    dve_ops.OPS.append(op)
    dve_ops.CUSTOM_DVE_SPECS[op.name] = spec
    dve_ops._SUB_OPCODE_FOR_NAME[op.name] = row
    return op


def _build_program():
    import concourse.bass as bass
    import concourse.bacc as bacc
    import concourse.mybir as mybir
    from concourse.tile import TileContext
    from bass_rust import VecI64Pair

    mul_scan = _register_mul_scan2x()

    f32 = mybir.dt.float32
    f16 = mybir.dt.float16

    def apv(base_ap, offset, dims):
        a = base_ap.copy()
        part = list(a.ap[0])
        a.ap = VecI64Pair([part] + [list(d) for d in dims])
        a.offset = a.offset + offset
        return a

    nc = bacc.Bacc()
    # per-core inputs (fp16): x slab [d,c,h,w]; y slab pre-shifted+padded,
    # channel-permuted by sigma(i)=25i%32: [d, i, 130, 130]
    x_in = nc.dram_tensor("xin", [D_LOC, C, H, W], f16, kind="ExternalInput")
    y_in = nc.dram_tensor("yin", [D_LOC, 128, 3 * C * WPAD], f16,
                          kind="ExternalInput")
    a1_in = nc.dram_tensor("a1in", [D_LOC, 128, 3 * C * 128], f16,
                           kind="ExternalInput")
    ident = nc.dram_tensor("ident", [32, 32], f16, kind="ExternalInput")
    out = nc.dram_tensor("out", [D_LOC, 128, 1152], f16,
                         kind="ExternalOutput")

    with TileContext(nc) as tc:
        with tc.tile_pool(name="const", bufs=1) as cpool, \
             tc.tile_pool(name="a", bufs=2) as apool, \
             tc.tile_pool(name="a1", bufs=2) as a1pool, \
             tc.tile_pool(name="xn", bufs=1) as xpool, \
             tc.tile_pool(name="xt", bufs=2) as xtpool, \
             tc.tile_pool(name="os", bufs=2) as ospool, \
             tc.tile_pool(name="ps", bufs=4, space="PSUM") as pspool:

            idt = cpool.tile([32, 32], f16)
            nc.sync.dma_start(idt[:], ident[:])

            # scan scratch: 2 banks x 3 kw-chunks of SCRW (banks alternate by
            # kh so scans of kh+1 never wait on diffs of kh); cols
            # bank*3*SCRW + kw*SCRW + {0,1} stay zero
            SCR = cpool.tile([128, 6 * SCRW], f16)
            zc = apv(SCR[:], 0, [[SCRW, 6], [1, 2]])
            nc.gpsimd.memset(zc, 0.0)

            def emit_loads(d):
                # x slice load first, in quarters: PE transposes of
                # quarter q start as soon as that quarter lands
                xf = xpool.tile([C, H * W], f16)
                for q in range(4):
                    nc.sync.dma_start(
                        xf[:, 4096 * q:4096 * (q + 1)],
                        x_in[d, :, 32 * q:32 * (q + 1), :].rearrange(
                            "c h w -> c (h w)"))
                # A[p, kh*4160 + i*130 + w] = y_pad[sigma(i), d, p+kh, w]
                # and its host-shifted kw=1 twin A1; both pre-arranged on
                # host, loaded in per-kh pieces so kh=0 scans start early
                A = apool.tile([128, 3 * C * WPAD], f16)
                A1 = a1pool.tile([128, 3 * C * 128], f16)
                for kh in range(3):
                    nc.sync.dma_start(
                        A[:, kh * C * WPAD:(kh + 1) * C * WPAD],
                        y_in[d, :, kh * C * WPAD:(kh + 1) * C * WPAD])
                    nc.sync.dma_start(
                        A1[:, kh * C * 128:(kh + 1) * C * 128],
                        a1_in[d, :, kh * C * 128:(kh + 1) * C * 128])
                return xf, A, A1

            def emit_build(d, xf, A):
                # XTW via PE transposes; PSUM -> SBUF fp16 via ACT
                XTW = xtpool.tile([128, NXT * 128], f16)
                for q in range(4):
                    for b in range(2):
                        PT = pspool.tile([128, 512], f16)
                        for mi in range(4):
                            m = 8 * q + 4 * b + mi
                            for j in range(4):
                                tin = apv(xf[:], 4 * m * 128 + j,
                                          [[128, 4], [4, 32]])
                                nc.tensor.transpose(
                                    PT[:, mi * 128 + j * 32:
                                       mi * 128 + (j + 1) * 32], tin, idt[:])
                        m0 = 8 * q + 4 * b
                        nc.scalar.copy(
                            XTW[:, m0 * 128:(m0 + 4) * 128], PT[:])
                        if m0 < 8:  # dup slots 32..39 <- m 0..7
                            nc.scalar.copy(
                                XTW[:, (32 + m0) * 128:(32 + m0 + 4) * 128],
                                PT[:])
                return XTW

            def emit_compute(d, A, A1, XTW):
                OS = ospool.tile([128, 1152], f16)
                for kh in range(3):
                    bank = ((d * 3 + kh) % 2) * 3 * SCRW
                    for kw in (0, 2, 1):
                        if kw == 1:
                            in0 = apv(A1[:], kh * C * 128,
                                      [[128, C], [1, 128]])
                        else:
                            in0 = apv(A[:], kh * C * WPAD + kw,
                                      [[WPAD, C], [1, 128]])
                        in1 = apv(XTW[:], (3 * kh + kw) * 128,
                                  [[128, C], [1, 128]])
                        o = apv(SCR[:], bank + kw * SCRW + 2, [[1, 4096]])
                        inst = nc.vector._custom_dve(mul_scan, out=o,
                                                     in0=in0, in1=in1)
                        if USE_2X:
                            inst.ins.perf_max = 1
                    # group sums = prefix diffs; ends chunk+33+32g, starts
                    # chunk+1+32g (col chunk+1 is the memset zero column)
                    ends = apv(SCR[:], bank + 33,
                               [[SCRW, 3], [128, C], [32, 4]])
                    starts = apv(SCR[:], bank + 1,
                                 [[SCRW, 3], [128, C], [32, 4]])
                    od = apv(OS[:], kh * 12, [[4, 3], [36, C], [1, 4]])
                    nc.vector.tensor_tensor(od, ends, starts,
                                            mybir.AluOpType.subtract)
                nc.sync.dma_start(out[d], OS[:])

            pending = None
            for d in range(D_LOC):
                xf, A, A1 = emit_loads(d)
                XTW = emit_build(d, xf, A)
                if pending is not None:
                    emit_compute(*pending)
                pending = (d, A, A1, XTW)
            emit_compute(*pending)

    nc.finalize()
    return nc


def _get_program():
    if "nc" not in _PROG_CACHE:
        _PROG_CACHE["nc"] = _build_program()
    return _PROG_CACHE["nc"]


_SIGMA = [(25 * i) % 32 for i in range(32)]      # channel stored at block i
_INV9 = [(9 * cp) % 32 for cp in range(32)]      # i such that sigma(i)=cp


def kernel(x: np.ndarray, y: np.ndarray) -> np.ndarray:
    from concourse.bass_utils import run_bass_kernel_spmd

    x = np.asarray(x)
    y = np.asarray(y)
    B, C_, D, H_, W_ = x.shape
    assert (B, C_, D, H_, W_) == (1, 32, 32, 128, 128)

    # host prep: fp16, depth-shifted + H/W-padded + sigma-permuted y,
    # pre-arranged into the on-chip A layout [d, p, kh*4160 + i*130 + w]
    y_sp = np.zeros((D, C_, WPAD, WPAD), np.float16)
    y_perm = y[0].transpose(1, 0, 2, 3)[:, _SIGMA]      # [d, i, h, w]
    y_sp[1:, :, 1:129, 1:129] = y_perm[:-1].astype(np.float16)
    y_hp = y_sp.transpose(0, 2, 1, 3)                   # [d, hrow, i, w]
    y_A = np.empty((D, 128, 3, C_, WPAD), np.float16)
    for kh in range(3):
        y_A[:, :, kh] = y_hp[:, kh:kh + 128]
    y_A = y_A.reshape(D, 128, 3 * C_ * WPAD)
    y_A1 = np.ascontiguousarray(
        y_A.reshape(D, 128, 3, C_, WPAD)[..., 1:129]).reshape(
        D, 128, 3 * C_ * 128)
    x_d = np.ascontiguousarray(
        x[0].transpose(1, 0, 2, 3).astype(np.float16))  # [d, c, h, w]
    id_np = np.eye(32, dtype=np.float16)

    nc = _get_program()
    in_maps = [
        {"xin": x_d[4 * j:4 * j + 4],
         "yin": y_A[4 * j:4 * j + 4],
         "a1in": y_A1[4 * j:4 * j + 4],
         "ident": id_np}
        for j in range(N_CORES)
    ]
    res = run_bass_kernel_spmd(nc, in_maps, core_ids=list(range(N_CORES)),
                               trace=_RUN_OPTS["trace"])
    _LAST_RESULT["res"] = res
    packed = np.concatenate([res.results[j]["out"] for j in range(N_CORES)],
                            axis=0).astype(np.float32)  # [32, 128, 1152]

    # host unpermute [d, p=(t,wb), i*36+k*4+j] -> [1, 9, D, H, W] + leaky
    a = packed.reshape(D, 4, 32, 32, 9, 4)               # d t wb i k j
    a = a[:, :, :, _INV9]                                # d t wb cp k j
    a = a.transpose(3, 4, 0, 1, 2, 5)                    # cp k d t wb j
    a = np.ascontiguousarray(a).reshape(9, 32, D, 4, 32, 4)  # k2 m d t wb j
    a = a.transpose(0, 2, 1, 3, 4, 5)                    # k2 d m t wb j
    a = np.ascontiguousarray(a).reshape(9, D, 128, 128)
    out = a[None]
    return np.where(out >= 0, out, np.float32(0.2) * out).astype(np.float32)
